# revision 23
# baseline (speedup 1.0000x reference)
"""Differential attention kernel for 8 Trainium2 NeuronCores.

Sharding: batch x head-group. Core c handles batch b = c//4, heads
[4g, 4g+4) with g = c%4. Each core computes Q/K/V projections for its
heads over the full sequence, causal differential attention, and its
partial O-projection; the host sums the 4 bf16 partials per batch.

Differential attention trick: score = (q1.k1 - lam*q2.k2) * scale is a
single K=128 matmul with stacked [q1*scale; -lam*scale*q2] and [k1; k2]
head vectors (scales folded into the projection weights on the host).

Softmax: scores are computed transposed (keys on partitions, queries
free), exp'd without max subtraction (inputs are bounded; exp is exact
to 2ULP on ACT), and the denominator comes for free from a ones-column
in V in the P@V matmul. Causality is applied structurally (upper blocks
skipped, diagonal blocks column-trimmed in the score matmul and zeroed
post-exp), which the host validates against the attention_mask input
before dispatch.

Emission order is engineered for engine overlap (the Tile scheduler is
a per-engine priority heap, priority = emission order, and PSUM pools
must coexist within 8 banks):
  A : QK proj heads 0,1        (pp 2x[128,2048] = 8 banks, DMA-paced)
  A2: V proj (3 banks) || QK proj head 2 (pp2 2x[128,512] = 2 banks)
  B : attention heads 0,1 (sc 4 + av 2 banks) || QK proj head 3 (2)
      - head-3 matmuls fill PE idle while ACT runs the softmax exp
  C : attention heads 2,3 || O-proj one chunk behind (op 2 banks)
"""
import math
from contextlib import ExitStack

import numpy as np
import ml_dtypes

S = 2048
H = 2048
NH = 16
HD = 64
NHC = 4          # heads per core
BF = ml_dtypes.bfloat16

_CACHED_NC = None


def _build_nc():
    import concourse.mybir as mybir
    import concourse.tile as tile
    from concourse import bacc

    bf16 = mybir.dt.bfloat16
    f32 = mybir.dt.float32
    AF = mybir.ActivationFunctionType

    nc = bacc.Bacc(None, target_bir_lowering=False)
    hT = nc.declare_dram_parameter("hT", [H, S], bf16, isOutput=False)
    wq = nc.declare_dram_parameter("wq", [H, NHC * 128], bf16, isOutput=False)
    wk = nc.declare_dram_parameter("wk", [H, NHC * 128], bf16, isOutput=False)
    wv = nc.declare_dram_parameter("wv", [H, NHC * 65], bf16, isOutput=False)
    wo = nc.declare_dram_parameter("wo", [128, 2, S], bf16, isOutput=False)
    out = nc.declare_dram_parameter("out", [S, H], bf16, isOutput=True)

    KT = H // 128    # 16 contraction tiles for projections
    NQ = S // 512    # 4 query chunks
    NS = S // 128    # 16 seq tiles

    with tile.TileContext(nc) as tc:
        with ExitStack() as ctx:
            # ---- persistent SBUF ----
            sb = ctx.enter_context(tc.tile_pool(name="sb", bufs=1))
            qk_sb = ctx.enter_context(tc.tile_pool(name="qk", bufs=1))
            ht_all = sb.tile([128, KT, S], bf16)          # hidden^T
            wq_sb = sb.tile([128, KT, NHC * 128], bf16)
            wk_sb = sb.tile([128, KT, NHC * 128], bf16)
            wv_sb = sb.tile([128, KT, NHC * 65], bf16)
            wo_sb = sb.tile([128, 2, S], bf16)            # head-pair stacked Wo rows
            qT = qk_sb.tile([128, NHC, S], bf16)          # [q1*s; -lam*s*q2] per head
            kT = qk_sb.tile([128, NHC, S], bf16)          # [k1; k2] per head
            v4 = qk_sb.tile([128, NS, NHC * 65], bf16)    # V tiles + ones cols
            avt = qk_sb.tile([128, 2, S], bf16)           # attn_out^T, head pairs stacked
            # warm the ACT exp table while DMAs stream in
            warm = sb.tile([1, 16], f32)
            nc.vector.memset(warm[:], 0.0)
            nc.scalar.activation(warm[:], warm[:], AF.Exp)
            # warm-up matmuls on scratch data: keep the PE busy through
            # the input-DMA ramp so HAM un-throttles (1.2 -> 2.4 GHz)
            # before the first real projection matmul issues
            wsrc = sb.tile([128, 512], bf16)
            nc.vector.memset(wsrc[:], 0.0)
            with tc.tile_pool(name="wmp", bufs=1, space="PSUM") as wmp:
                wps = wmp.tile([128, 512], f32)
                for _ in range(36):
                    nc.tensor.matmul(wps[:], lhsT=wsrc[:, 0:128], rhs=wsrc[:],
                                     start=True, stop=True)

            # ---- input DMAs, k-ordered so the phase-A chains (k-outer)
            # start as soon as the first rows land. hT alternates across
            # both HWDGE queues (phase A is paced by hT arrival: each
            # 512 KB tile backs 1.9 us of PE work, one queue sustains
            # only ~175 GB/s). wq/wk ride the scalar queue; wv/wo are
            # only needed from phase A2 on and go on the slower gpsimd
            # SWDGE queue. ----
            nc.scalar.dma_start(out=wq_sb[:, 0, :], in_=wq[0:128, :])
            nc.scalar.dma_start(out=wk_sb[:, 0, :], in_=wk[0:128, :])
            for k in range(KT):
                # two column strips per tile, one per HWDGE queue, so the
                # first matmuls of each k-step aren't gated on the full
                # 512 KB tile landing
                rs = slice(k * 128, (k + 1) * 128)
                nc.sync.dma_start(out=ht_all[:, k, 0:1024], in_=hT[rs, 0:1024])
                nc.scalar.dma_start(out=ht_all[:, k, 1024:2048], in_=hT[rs, 1024:2048])
                if k > 0:
                    nc.gpsimd.dma_start(out=wq_sb[:, k, :], in_=wq[rs, :])
                    nc.gpsimd.dma_start(out=wk_sb[:, k, :], in_=wk[rs, :])
            for k in range(KT):
                nc.gpsimd.dma_start(out=wv_sb[:, k, :],
                                    in_=wv[k * 128:(k + 1) * 128, :])
            nc.gpsimd.dma_start(out=wo_sb[:], in_=wo[:, :, :])

            # ---- phase A: q/k projections for heads 0,1 (k-outer,
            # nj-inner: each k-step is 4x512 cols so the PE keeps pace
            # with the hT DMA stream) ----
            with tc.tile_pool(name="pjp", bufs=2, space="PSUM") as pjp:
                for h in range(2):
                    hs = slice(h * 128, (h + 1) * 128)
                    for w_sb, dst in ((wq_sb, qT), (wk_sb, kT)):
                        pp = pjp.tile([128, S], f32, tag="qkp")
                        for k in range(KT):
                            for nj in range(NQ):
                                nc.tensor.matmul(pp[:, nj * 512:(nj + 1) * 512],
                                                 lhsT=w_sb[:, k, hs],
                                                 rhs=ht_all[:, k, nj * 512:(nj + 1) * 512],
                                                 start=(k == 0), stop=(k == KT - 1))
                        for nj in range(NQ):
                            nc.vector.tensor_copy(dst[:, h, nj * 512:(nj + 1) * 512],
                                                  pp[:, nj * 512:(nj + 1) * 512])

            # 2-bank PSUM pool shared (same tag -> same slots, PSUM pool
            # lifetimes are LIFO) by the head-2/3 projection chains and
            # later by the O-projection tiles
            ch5 = ctx.enter_context(tc.tile_pool(name="ch5", bufs=2, space="PSUM"))

            # small-footprint q/k projection chain for heads 2,3 (hT is
            # fully resident by the time these run, so no DMA pacing
            # concerns; 1 PSUM bank per chain)
            def qk23_chain(h, w_sb, dst, nj):
                hs = slice(h * 128, (h + 1) * 128)
                qs = slice(nj * 512, (nj + 1) * 512)
                pq = ch5.tile([128, 512], f32, tag="ch512")
                for k in range(KT):
                    nc.tensor.matmul(pq[:], lhsT=w_sb[:, k, hs],
                                     rhs=ht_all[:, k, qs],
                                     start=(k == 0), stop=(k == KT - 1))
                nc.vector.tensor_copy(dst[:, h, qs], pq[:])

            # ---- phase A2: V projection (natural [S, 260] layout)
            # interleaved with head-2 q/k chains ----
            h2_chains = [(2, w_sb, dst, nj)
                         for w_sb, dst in ((wq_sb, qT), (wk_sb, kT))
                         for nj in range(NQ)]
            with tc.tile_pool(name="vjp", bufs=3, space="PSUM") as vjp:
                for st in range(NS):
                    vp = vjp.tile([128, NHC * 65], f32, tag="vp")
                    for k in range(KT):
                        nc.tensor.matmul(vp[:], lhsT=ht_all[:, k, st * 128:(st + 1) * 128],
                                         rhs=wv_sb[:, k, :],
                                         start=(k == 0), stop=(k == KT - 1))
                    nc.vector.tensor_copy(v4[:, st, :], vp[:])
                    for j in range(NHC):
                        nc.gpsimd.memset(v4[:, st, j * 65 + 64:j * 65 + 65], 1.0)
                    if st % 2 == 1:
                        h2_chains and qk23_chain(*h2_chains.pop(0))

            # ---- attention chunk for one head pair ----
            att_work = ctx.enter_context(tc.tile_pool(name="attw", bufs=3))
            nrm_work = ctx.enter_context(tc.tile_pool(name="nrmw", bufs=2))
            atp = ctx.enter_context(tc.tile_pool(name="atp", bufs=1, space="PSUM"))

            def attn_chunk(pair, nj):
                """Scores + exp + P@V for heads (2*pair, 2*pair+1) on
                query chunk nj. Returns the two av PSUM tiles."""
                qs = slice(nj * 512, (nj + 1) * 512)
                nblk = 4 * nj + 4
                pair_heads = (2 * pair, 2 * pair + 1)
                pav = {}
                for h in pair_heads:
                    pav[h] = atp.tile([65, 512], f32, tag=f"av{h % 2}",
                                      name=f"av{h}")
                # round-robin the two heads per key-block group so one
                # head's score matmuls hide the other head's exp
                for kg in range(nblk // 2):
                    scs, pts = {}, {}
                    # per-u start column: diagonal blocks (uu>=0) have
                    # their first 128*uu query columns fully masked, so
                    # both the score matmul and the exp skip them
                    cuts = [max(2 * kg + u - 4 * nj, 0) * 128 for u in range(2)]
                    for h in pair_heads:
                        sc = atp.tile([128, 1024], f32, tag=f"sc{h % 2}",
                                      name=f"sc{h}")
                        scs[h] = sc
                        for u in range(2):
                            ki = 2 * kg + u
                            c0 = cuts[u]
                            nc.tensor.matmul(sc[:, u * 512 + c0:(u + 1) * 512],
                                             lhsT=kT[:, h, ki * 128:(ki + 1) * 128],
                                             rhs=qT[:, h, nj * 512 + c0:(nj + 1) * 512],
                                             start=True, stop=True)
                    for h in pair_heads:
                        pt = att_work.tile([128, 1024], bf16, tag=f"pt{h % 2}",
                                           bufs=4, name=f"pt{h}")
                        pts[h] = pt
                        if cuts[1] == 0:
                            nc.scalar.activation(pt[:, cuts[0]:1024],
                                                 scs[h][:, cuts[0]:1024], AF.Exp)
                        else:
                            for u in range(2):
                                lo, hi = u * 512 + cuts[u], (u + 1) * 512
                                nc.scalar.activation(pt[:, lo:hi],
                                                     scs[h][:, lo:hi], AF.Exp)
                    for h in pair_heads:
                        pt = pts[h]
                        av = pav[h]
                        for u in range(2):
                            ki = 2 * kg + u
                            uu = ki - 4 * nj  # >=0 on diagonal blocks
                            if uu >= 0:
                                b0 = u * 512 + uu * 128
                                nc.gpsimd.affine_select(
                                    out=pt[:, b0:b0 + 128],
                                    in_=pt[:, b0:b0 + 128],
                                    compare_op=mybir.AluOpType.is_ge,
                                    fill=0.0,
                                    base=0,
                                    channel_multiplier=-1,
                                    pattern=[[1, 128]],
                                )
                                nc.tensor.matmul(av[:, uu * 128:512],
                                                 lhsT=v4[:, ki, h * 65:(h + 1) * 65],
                                                 rhs=pt[:, u * 512 + uu * 128:(u + 1) * 512],
                                                 start=(ki == 0), stop=(ki == nblk - 1))
                            else:
                                nc.tensor.matmul(av[:],
                                                 lhsT=v4[:, ki, h * 65:(h + 1) * 65],
                                                 rhs=pt[:, u * 512:(u + 1) * 512],
                                                 start=(ki == 0), stop=(ki == nblk - 1))
                return pav

            def norm_chunk(pair, nj, pav):
                """row 64 (even) / 63 (odd) of av is the softmax
                denominator; scale the 64 attn rows by 1/denom into avt.
                Reciprocal runs on a [128, 8] DMA-folded layout so all
                128 DVE lanes work instead of one."""
                qs = slice(nj * 512, (nj + 1) * 512)
                pair_heads = (2 * pair, 2 * pair + 1)
                dfold = nrm_work.tile([128, 8], f32, tag="dfold")
                araws = {}
                for j, h in enumerate(pair_heads):
                    den = nrm_work.tile([65, 512], f32, tag="den", bufs=4)
                    nc.vector.tensor_copy(den[64:65, :], pav[h][64:65, :])
                    nc.sync.dma_start(out=dfold[:, 4 * j:4 * j + 4], in_=den[64:65, :])
                    # evacuate the numerator too so the PSUM bank frees
                    # without waiting for the normalization chain
                    araw = nrm_work.tile([64, 512], bf16, tag=f"araw{h % 2}",
                                         bufs=2, name=f"araw{h}")
                    nc.vector.tensor_copy(araw[:], pav[h][0:64, :])
                    araws[h] = araw
                nc.vector.reciprocal(dfold[:], dfold[:])
                for j, h in enumerate(pair_heads):
                    rc0 = nrm_work.tile([1, 512], f32, tag="rc0")
                    nc.sync.dma_start(out=rc0[:], in_=dfold[:, 4 * j:4 * j + 4])
                    bcs = nrm_work.tile([64, 512], f32, tag=f"bcs{h % 2}",
                                        name=f"bcs{h}")
                    nc.gpsimd.partition_broadcast(bcs[:], rc0[:])
                    if h % 2:
                        om = nrm_work.tile([64, 512], bf16, tag="om", bufs=2)
                        nc.vector.tensor_mul(om[:], araws[h][:], bcs[:])
                        nc.sync.dma_start(out=avt[64:128, pair, qs], in_=om[:])
                    else:
                        nc.vector.tensor_mul(avt[0:64, pair, qs], araws[h][:], bcs[:])

            # ---- phase B: attention pair 0, head-3 q/k chains woven in
            # between chunks (they fill PE idle while ACT runs exp) ----
            h3_chains = [(3, w_sb, dst, nj)
                         for w_sb, dst in ((wq_sb, qT), (wk_sb, kT))
                         for nj in range(NQ)]
            for nj in range(NQ):
                pav = attn_chunk(0, nj)
                norm_chunk(0, nj, pav)
                for _ in range(2):
                    h3_chains and qk23_chain(*h3_chains.pop(0))

            # ---- phase C: attention pair 1 with O-projection trailing
            # one chunk behind ----
            oout_sb = ctx.enter_context(tc.tile_pool(name="oout", bufs=4))

            def oproj_chunk(nj):
                for qi in range(4 * nj, 4 * nj + 4):
                    for nch in range(NQ):
                        op = ch5.tile([128, 512], f32, tag="ch512")
                        for p in range(2):
                            nc.tensor.matmul(op[:],
                                             lhsT=avt[:, p, qi * 128:(qi + 1) * 128],
                                             rhs=wo_sb[:, p, nch * 512:(nch + 1) * 512],
                                             start=(p == 0), stop=(p == 1))
                        ot = oout_sb.tile([128, 512], bf16, tag="ot")
                        # offload 1-in-4 evacuations to ACT: DVE is the
                        # busiest engine in phase C but ACT carries the
                        # exps, so only skim its spare duty
                        if nch % 4 == 1:
                            nc.scalar.copy(ot[:], op[:])
                        else:
                            nc.vector.tensor_copy(ot[:], op[:])
                        nc.sync.dma_start(
                            out=out[qi * 128:(qi + 1) * 128, nch * 512:(nch + 1) * 512],
                            in_=ot[:])

            for nj in range(NQ):
                pav = attn_chunk(1, nj)
                norm_chunk(1, nj, pav)
                if nj > 0:
                    oproj_chunk(nj - 1)
            oproj_chunk(NQ - 1)
    return nc


def _get_nc():
    global _CACHED_NC
    if _CACHED_NC is None:
        nc = _build_nc()
        if not nc.is_finalized():
            nc.finalize()
        _CACHED_NC = nc
    return _CACHED_NC


def _prep_in_maps(hidden_states, Wq, Wk, Wv, Wo, lambda_param):
    lam = math.tanh(math.log1p(math.exp(float(lambda_param))))
    scale = HD ** -0.5
    in_maps = []
    hTb = [np.ascontiguousarray(hidden_states[b].T).astype(BF) for b in range(2)]
    for core in range(8):
        b, g = divmod(core, 4)
        heads = range(NHC * g, NHC * g + NHC)
        wq_cols, wk_cols = [], []
        for h in heads:
            wq_cols.append(Wq[:, h * 64:(h + 1) * 64] * scale)
            wq_cols.append(Wq[:, (NH + h) * 64:(NH + h + 1) * 64] * (-lam * scale))
            wk_cols.append(Wk[:, h * 64:(h + 1) * 64])
            wk_cols.append(Wk[:, (NH + h) * 64:(NH + h + 1) * 64])
        wv_pad = np.zeros((H, NHC * 65), dtype=np.float32)
        for j, h in enumerate(heads):
            wv_pad[:, j * 65:j * 65 + 64] = Wv[:, h * 64:(h + 1) * 64]
        heads = list(heads)
        wo_sel = np.zeros((128, 2, S), dtype=np.float32)  # head-pair stacked rows
        for p in range(2):
            h0, h1 = heads[2 * p], heads[2 * p + 1]
            wo_sel[0:64, p] = Wo[h0 * 64:(h0 + 1) * 64, :]
            wo_sel[64:128, p] = Wo[h1 * 64:(h1 + 1) * 64, :]
        in_maps.append({
            "hT": hTb[b],
            "wq": np.concatenate(wq_cols, axis=1).astype(BF),
            "wk": np.concatenate(wk_cols, axis=1).astype(BF),
            "wv": wv_pad.astype(BF),
            "wo": np.ascontiguousarray(wo_sel).astype(BF),
        })
    return in_maps


def _mask_is_causal(attention_mask):
    m = np.asarray(attention_mask)
    if m.shape != (2, 1, S, S):
        return False
    neg = np.float32(np.finfo(np.float32).min)
    tri = np.tril(np.ones((S, S), dtype=bool))
    expect = np.where(tri, np.float32(0.0), neg)
    return all(np.array_equal(m[b, 0], expect) for b in range(m.shape[0]))


def _fallback(hidden_states, attention_mask, Wq, Wk, Wv, Wo, lambda_param):
    hs = hidden_states.astype(np.float32)
    lam = math.tanh(math.log1p(math.exp(float(lambda_param))))
    scaling = HD ** -0.5
    B = hs.shape[0]
    out = np.empty((B, S, H), dtype=np.float32)
    for b in range(B):
        q_all = (hs[b] @ Wq).reshape(S, 2 * NH, HD).transpose(1, 0, 2)
        k_all = (hs[b] @ Wk).reshape(S, 2 * NH, HD).transpose(1, 0, 2)
        v = (hs[b] @ Wv).reshape(S, NH, HD).transpose(1, 0, 2)
        acc = np.zeros((S, H), dtype=np.float32)
        for h in range(NH):
            s1 = q_all[h] @ k_all[h].T
            s2 = q_all[NH + h] @ k_all[NH + h].T
            sc = (s1 - lam * s2) * scaling + attention_mask[b, 0]
            sc -= sc.max(axis=-1, keepdims=True)
            p = np.exp(sc)
            p /= p.sum(axis=-1, keepdims=True)
            acc += (p @ v[h]) @ Wo[h * 64:(h + 1) * 64]
        out[b] = acc
    return out


def _run(inputs, trace=False):
    from concourse.bass_utils import run_bass_kernel_spmd

    hidden_states = np.asarray(inputs["hidden_states"], dtype=np.float32)
    attention_mask = np.asarray(inputs["attention_mask"], dtype=np.float32)
    Wq = np.asarray(inputs["Wq"], dtype=np.float32)
    Wk = np.asarray(inputs["Wk"], dtype=np.float32)
    Wv = np.asarray(inputs["Wv"], dtype=np.float32)
    Wo = np.asarray(inputs["Wo"], dtype=np.float32)
    lam_p = inputs["lambda_param"]

    if not _mask_is_causal(attention_mask):
        return _fallback(hidden_states, attention_mask, Wq, Wk, Wv, Wo, lam_p), None

    in_maps = _prep_in_maps(hidden_states, Wq, Wk, Wv, Wo, lam_p)
    nc = _get_nc()
    res = run_bass_kernel_spmd(nc, in_maps, list(range(8)), trace=trace)
    out = np.empty((2, S, H), dtype=np.float32)
    for b in range(2):
        acc = res.results[4 * b]["out"].astype(np.float32)
        for g in range(1, 4):
            acc = acc + res.results[4 * b + g]["out"].astype(np.float32)
        out[b] = acc
    return out, res


def kernel(**inputs):
    out, _ = _run(inputs, trace=False)
    return out


# revision 27
# speedup vs baseline: 1.0046x; 1.0046x over previous
"""Differential attention kernel for 8 Trainium2 NeuronCores.

Sharding: batch x head-group. Core c handles batch b = c//4, heads
[4g, 4g+4) with g = c%4. Each core computes Q/K/V projections for its
heads over the full sequence, causal differential attention, and its
partial O-projection; the host sums the 4 bf16 partials per batch.

Differential attention trick: score = (q1.k1 - lam*q2.k2) * scale is a
single K=128 matmul with stacked [q1*scale; -lam*scale*q2] and [k1; k2]
head vectors (scales folded into the projection weights on the host).

Softmax: scores are computed transposed (keys on partitions, queries
free), exp'd without max subtraction (inputs are bounded; exp is exact
to 2ULP on ACT), and the denominator comes for free from a ones-column
in V in the P@V matmul. Causality is applied structurally (upper blocks
skipped, diagonal blocks column-trimmed in the score matmul and zeroed
post-exp), which the host validates against the attention_mask input
before dispatch.

Emission order is engineered for engine overlap (the Tile scheduler is
a per-engine priority heap, priority = emission order, and PSUM pools
must coexist within 8 banks):
  A : QK proj heads 0,1        (pp 2x[128,2048] = 8 banks, DMA-paced)
  A2: V proj (3 banks) || QK proj head 2 (pp2 2x[128,512] = 2 banks)
  B : attention heads 0,1 (sc 4 + av 2 banks) || QK proj head 3 (2)
      - head-3 matmuls fill PE idle while ACT runs the softmax exp
  C : attention heads 2,3 || O-proj one chunk behind (op 2 banks)
"""
import math
from contextlib import ExitStack

import numpy as np
import ml_dtypes

S = 2048
H = 2048
NH = 16
HD = 64
NHC = 4          # heads per core
BF = ml_dtypes.bfloat16

_CACHED_NC = None


def _build_nc():
    import concourse.mybir as mybir
    import concourse.tile as tile
    from concourse import bacc

    bf16 = mybir.dt.bfloat16
    f32 = mybir.dt.float32
    AF = mybir.ActivationFunctionType

    nc = bacc.Bacc(None, target_bir_lowering=False)
    hT = nc.declare_dram_parameter("hT", [H, S], bf16, isOutput=False)
    wq = nc.declare_dram_parameter("wq", [H, NHC * 128], bf16, isOutput=False)
    wk = nc.declare_dram_parameter("wk", [H, NHC * 128], bf16, isOutput=False)
    wv = nc.declare_dram_parameter("wv", [H, NHC * 65], bf16, isOutput=False)
    wo = nc.declare_dram_parameter("wo", [128, 2, S], bf16, isOutput=False)
    out = nc.declare_dram_parameter("out", [S, H], bf16, isOutput=True)

    KT = H // 128    # 16 contraction tiles for projections
    NQ = S // 512    # 4 query chunks
    NS = S // 128    # 16 seq tiles

    with tile.TileContext(nc) as tc:
        with ExitStack() as ctx:
            # ---- persistent SBUF ----
            sb = ctx.enter_context(tc.tile_pool(name="sb", bufs=1))
            qk_sb = ctx.enter_context(tc.tile_pool(name="qk", bufs=1))
            ht_all = sb.tile([128, KT, S], bf16)          # hidden^T
            wq_sb = sb.tile([128, KT, NHC * 128], bf16)
            wk_sb = sb.tile([128, KT, NHC * 128], bf16)
            wv_sb = sb.tile([128, KT, NHC * 65], bf16)
            wo_sb = sb.tile([128, 2, S], bf16)            # head-pair stacked Wo rows
            qT = qk_sb.tile([128, NHC, S], bf16)          # [q1*s; -lam*s*q2] per head
            kT = qk_sb.tile([128, NHC, S], bf16)          # [k1; k2] per head
            v4 = qk_sb.tile([128, NS, NHC * 65], bf16)    # V tiles + ones cols
            avt = qk_sb.tile([128, 2, S], bf16)           # attn_out^T, head pairs stacked
            # warm the ACT exp table while DMAs stream in
            warm = sb.tile([1, 16], f32)
            nc.vector.memset(warm[:], 0.0)
            nc.scalar.activation(warm[:], warm[:], AF.Exp)
            # warm-up matmuls on scratch data: keep the PE busy through
            # the input-DMA ramp so HAM un-throttles (1.2 -> 2.4 GHz)
            # before the first real projection matmul issues
            wsrc = sb.tile([128, 512], bf16)
            nc.vector.memset(wsrc[:], 0.0)
            with tc.tile_pool(name="wmp", bufs=1, space="PSUM") as wmp:
                wps = wmp.tile([128, 512], f32)
                for _ in range(20):
                    nc.tensor.matmul(wps[:], lhsT=wsrc[:, 0:128], rhs=wsrc[:],
                                     start=True, stop=True)

            # ---- input DMAs, k-ordered so the phase-A chains (k-outer)
            # start as soon as the first rows land. hT alternates across
            # both HWDGE queues (phase A is paced by hT arrival: each
            # 512 KB tile backs 1.9 us of PE work, one queue sustains
            # only ~175 GB/s). wq/wk ride the scalar queue; wv/wo are
            # only needed from phase A2 on and go on the slower gpsimd
            # SWDGE queue. ----
            nc.scalar.dma_start(out=wq_sb[:, 0, :], in_=wq[0:128, :])
            nc.scalar.dma_start(out=wk_sb[:, 0, :], in_=wk[0:128, :])
            for k in range(KT):
                # two column strips per tile, one per HWDGE queue, so the
                # first matmuls of each k-step aren't gated on the full
                # 512 KB tile landing
                rs = slice(k * 128, (k + 1) * 128)
                nc.sync.dma_start(out=ht_all[:, k, 0:1024], in_=hT[rs, 0:1024])
                nc.scalar.dma_start(out=ht_all[:, k, 1024:2048], in_=hT[rs, 1024:2048])
                if k > 0:
                    nc.scalar.dma_start(out=wq_sb[:, k, :], in_=wq[rs, :])
                    nc.sync.dma_start(out=wk_sb[:, k, :], in_=wk[rs, :])
                nc.gpsimd.dma_start(out=wv_sb[:, k, :], in_=wv[rs, :])
            nc.gpsimd.dma_start(out=wo_sb[:], in_=wo[:, :, :])

            # ---- phase A: q/k projections for heads 0,1 (k-outer,
            # nj-inner: each k-step is 4x512 cols so the PE keeps pace
            # with the hT DMA stream) ----
            with tc.tile_pool(name="pjp", bufs=2, space="PSUM") as pjp:
                for h in range(2):
                    hs = slice(h * 128, (h + 1) * 128)
                    for w_sb, dst in ((wq_sb, qT), (wk_sb, kT)):
                        pp = pjp.tile([128, S], f32, tag="qkp")
                        for k in range(KT):
                            for nj in range(NQ):
                                nc.tensor.matmul(pp[:, nj * 512:(nj + 1) * 512],
                                                 lhsT=w_sb[:, k, hs],
                                                 rhs=ht_all[:, k, nj * 512:(nj + 1) * 512],
                                                 start=(k == 0), stop=(k == KT - 1))
                        for nj in range(NQ):
                            nc.vector.tensor_copy(dst[:, h, nj * 512:(nj + 1) * 512],
                                                  pp[:, nj * 512:(nj + 1) * 512])

            # 2-bank PSUM pool shared (same tag -> same slots, PSUM pool
            # lifetimes are LIFO) by the head-2/3 projection chains and
            # later by the O-projection tiles
            ch5 = ctx.enter_context(tc.tile_pool(name="ch5", bufs=2, space="PSUM"))

            # small-footprint q/k projection chain for heads 2,3 (hT is
            # fully resident by the time these run, so no DMA pacing
            # concerns; 1 PSUM bank per chain)
            def qk23_chain(h, w_sb, dst, nj):
                hs = slice(h * 128, (h + 1) * 128)
                qs = slice(nj * 512, (nj + 1) * 512)
                pq = ch5.tile([128, 512], f32, tag="ch512")
                for k in range(KT):
                    nc.tensor.matmul(pq[:], lhsT=w_sb[:, k, hs],
                                     rhs=ht_all[:, k, qs],
                                     start=(k == 0), stop=(k == KT - 1))
                nc.vector.tensor_copy(dst[:, h, qs], pq[:])

            # ---- phase A2: V projection (natural [S, 260] layout)
            # interleaved with head-2 q/k chains ----
            h2_chains = [(2, w_sb, dst, nj)
                         for w_sb, dst in ((wq_sb, qT), (wk_sb, kT))
                         for nj in range(NQ)]
            with tc.tile_pool(name="vjp", bufs=3, space="PSUM") as vjp:
                for st in range(NS):
                    vp = vjp.tile([128, NHC * 65], f32, tag="vp")
                    for k in range(KT):
                        nc.tensor.matmul(vp[:], lhsT=ht_all[:, k, st * 128:(st + 1) * 128],
                                         rhs=wv_sb[:, k, :],
                                         start=(k == 0), stop=(k == KT - 1))
                    nc.vector.tensor_copy(v4[:, st, :], vp[:])
                    for j in range(NHC):
                        nc.gpsimd.memset(v4[:, st, j * 65 + 64:j * 65 + 65], 1.0)
                    if st % 2 == 1:
                        h2_chains and qk23_chain(*h2_chains.pop(0))

            # ---- attention chunk for one head pair ----
            att_work = ctx.enter_context(tc.tile_pool(name="attw", bufs=3))
            nrm_work = ctx.enter_context(tc.tile_pool(name="nrmw", bufs=2))
            atp = ctx.enter_context(tc.tile_pool(name="atp", bufs=1, space="PSUM"))

            def attn_chunk(pair, nj):
                """Scores + exp + P@V for heads (2*pair, 2*pair+1) on
                query chunk nj. Returns the two av PSUM tiles."""
                qs = slice(nj * 512, (nj + 1) * 512)
                nblk = 4 * nj + 4
                pair_heads = (2 * pair, 2 * pair + 1)
                pav = {}
                for h in pair_heads:
                    pav[h] = atp.tile([65, 512], f32, tag=f"av{h % 2}",
                                      name=f"av{h}")
                # round-robin the two heads per key-block group so one
                # head's score matmuls hide the other head's exp
                for kg in range(nblk // 2):
                    scs, pts = {}, {}
                    # per-u start column: diagonal blocks (uu>=0) have
                    # their first 128*uu query columns fully masked, so
                    # both the score matmul and the exp skip them
                    cuts = [max(2 * kg + u - 4 * nj, 0) * 128 for u in range(2)]
                    for h in pair_heads:
                        sc = atp.tile([128, 1024], f32, tag=f"sc{h % 2}",
                                      name=f"sc{h}")
                        scs[h] = sc
                        for u in range(2):
                            ki = 2 * kg + u
                            c0 = cuts[u]
                            nc.tensor.matmul(sc[:, u * 512 + c0:(u + 1) * 512],
                                             lhsT=kT[:, h, ki * 128:(ki + 1) * 128],
                                             rhs=qT[:, h, nj * 512 + c0:(nj + 1) * 512],
                                             start=True, stop=True)
                    for h in pair_heads:
                        pt = att_work.tile([128, 1024], bf16, tag=f"pt{h % 2}",
                                           bufs=4, name=f"pt{h}")
                        pts[h] = pt
                        if cuts[1] == 0:
                            nc.scalar.activation(pt[:, cuts[0]:1024],
                                                 scs[h][:, cuts[0]:1024], AF.Exp)
                        else:
                            for u in range(2):
                                lo, hi = u * 512 + cuts[u], (u + 1) * 512
                                nc.scalar.activation(pt[:, lo:hi],
                                                     scs[h][:, lo:hi], AF.Exp)
                    for h in pair_heads:
                        pt = pts[h]
                        av = pav[h]
                        for u in range(2):
                            ki = 2 * kg + u
                            uu = ki - 4 * nj  # >=0 on diagonal blocks
                            if uu >= 0:
                                b0 = u * 512 + uu * 128
                                nc.gpsimd.affine_select(
                                    out=pt[:, b0:b0 + 128],
                                    in_=pt[:, b0:b0 + 128],
                                    compare_op=mybir.AluOpType.is_ge,
                                    fill=0.0,
                                    base=0,
                                    channel_multiplier=-1,
                                    pattern=[[1, 128]],
                                )
                                nc.tensor.matmul(av[:, uu * 128:512],
                                                 lhsT=v4[:, ki, h * 65:(h + 1) * 65],
                                                 rhs=pt[:, u * 512 + uu * 128:(u + 1) * 512],
                                                 start=(ki == 0), stop=(ki == nblk - 1))
                            else:
                                nc.tensor.matmul(av[:],
                                                 lhsT=v4[:, ki, h * 65:(h + 1) * 65],
                                                 rhs=pt[:, u * 512:(u + 1) * 512],
                                                 start=(ki == 0), stop=(ki == nblk - 1))
                return pav

            def norm_chunk(pair, nj, pav):
                """row 64 (even) / 63 (odd) of av is the softmax
                denominator; scale the 64 attn rows by 1/denom into avt.
                Reciprocal runs on a [128, 8] DMA-folded layout so all
                128 DVE lanes work instead of one."""
                qs = slice(nj * 512, (nj + 1) * 512)
                pair_heads = (2 * pair, 2 * pair + 1)
                dfold = nrm_work.tile([128, 8], f32, tag="dfold")
                araws = {}
                for j, h in enumerate(pair_heads):
                    den = nrm_work.tile([65, 512], f32, tag="den", bufs=4)
                    nc.vector.tensor_copy(den[64:65, :], pav[h][64:65, :])
                    nc.sync.dma_start(out=dfold[:, 4 * j:4 * j + 4], in_=den[64:65, :])
                    # evacuate the numerator too so the PSUM bank frees
                    # without waiting for the normalization chain
                    araw = nrm_work.tile([64, 512], bf16, tag=f"araw{h % 2}",
                                         bufs=2, name=f"araw{h}")
                    nc.vector.tensor_copy(araw[:], pav[h][0:64, :])
                    araws[h] = araw
                nc.vector.reciprocal(dfold[:], dfold[:])
                for j, h in enumerate(pair_heads):
                    rc0 = nrm_work.tile([1, 512], f32, tag="rc0")
                    nc.sync.dma_start(out=rc0[:], in_=dfold[:, 4 * j:4 * j + 4])
                    bcs = nrm_work.tile([64, 512], f32, tag=f"bcs{h % 2}",
                                        name=f"bcs{h}")
                    nc.gpsimd.partition_broadcast(bcs[:], rc0[:])
                    if h % 2:
                        om = nrm_work.tile([64, 512], bf16, tag="om", bufs=2)
                        nc.vector.tensor_mul(om[:], araws[h][:], bcs[:])
                        nc.sync.dma_start(out=avt[64:128, pair, qs], in_=om[:])
                    else:
                        nc.vector.tensor_mul(avt[0:64, pair, qs], araws[h][:], bcs[:])

            # ---- phase B: attention pair 0, head-3 q/k chains woven in
            # between chunks (they fill PE idle while ACT runs exp) ----
            h3_chains = [(3, w_sb, dst, nj)
                         for w_sb, dst in ((wq_sb, qT), (wk_sb, kT))
                         for nj in range(NQ)]
            for nj in range(NQ):
                pav = attn_chunk(0, nj)
                norm_chunk(0, nj, pav)
                for _ in range(2):
                    h3_chains and qk23_chain(*h3_chains.pop(0))

            # ---- phase C: attention pair 1 with O-projection trailing
            # one chunk behind ----
            oout_sb = ctx.enter_context(tc.tile_pool(name="oout", bufs=4))

            def oproj_chunk(nj, tail=False):
                for qi in range(4 * nj, 4 * nj + 4):
                    for nch in range(NQ):
                        op = ch5.tile([128, 512], f32, tag="ch512")
                        for p in range(2):
                            nc.tensor.matmul(op[:],
                                             lhsT=avt[:, p, qi * 128:(qi + 1) * 128],
                                             rhs=wo_sb[:, p, nch * 512:(nch + 1) * 512],
                                             start=(p == 0), stop=(p == 1))
                        ot = oout_sb.tile([128, 512], bf16, tag="ot")
                        # DVE is the busiest engine in phase C but ACT
                        # carries the exps, so only skim its spare duty
                        # (1-in-4) until the exps are done (tail: 1-in-2)
                        if nch % (2 if tail else 4) == 1:
                            nc.scalar.copy(ot[:], op[:])
                        else:
                            nc.vector.tensor_copy(ot[:], op[:])
                        nc.sync.dma_start(
                            out=out[qi * 128:(qi + 1) * 128, nch * 512:(nch + 1) * 512],
                            in_=ot[:])

            for nj in range(NQ):
                pav = attn_chunk(1, nj)
                norm_chunk(1, nj, pav)
                if nj > 0:
                    oproj_chunk(nj - 1)
            oproj_chunk(NQ - 1, tail=True)
    return nc


def _get_nc():
    global _CACHED_NC
    if _CACHED_NC is None:
        nc = _build_nc()
        if not nc.is_finalized():
            nc.finalize()
        _CACHED_NC = nc
    return _CACHED_NC


def _prep_in_maps(hidden_states, Wq, Wk, Wv, Wo, lambda_param):
    lam = math.tanh(math.log1p(math.exp(float(lambda_param))))
    scale = HD ** -0.5
    in_maps = []
    hTb = [np.ascontiguousarray(hidden_states[b].T).astype(BF) for b in range(2)]
    for core in range(8):
        b, g = divmod(core, 4)
        heads = range(NHC * g, NHC * g + NHC)
        wq_cols, wk_cols = [], []
        for h in heads:
            wq_cols.append(Wq[:, h * 64:(h + 1) * 64] * scale)
            wq_cols.append(Wq[:, (NH + h) * 64:(NH + h + 1) * 64] * (-lam * scale))
            wk_cols.append(Wk[:, h * 64:(h + 1) * 64])
            wk_cols.append(Wk[:, (NH + h) * 64:(NH + h + 1) * 64])
        wv_pad = np.zeros((H, NHC * 65), dtype=np.float32)
        for j, h in enumerate(heads):
            wv_pad[:, j * 65:j * 65 + 64] = Wv[:, h * 64:(h + 1) * 64]
        heads = list(heads)
        wo_sel = np.zeros((128, 2, S), dtype=np.float32)  # head-pair stacked rows
        for p in range(2):
            h0, h1 = heads[2 * p], heads[2 * p + 1]
            wo_sel[0:64, p] = Wo[h0 * 64:(h0 + 1) * 64, :]
            wo_sel[64:128, p] = Wo[h1 * 64:(h1 + 1) * 64, :]
        in_maps.append({
            "hT": hTb[b],
            "wq": np.concatenate(wq_cols, axis=1).astype(BF),
            "wk": np.concatenate(wk_cols, axis=1).astype(BF),
            "wv": wv_pad.astype(BF),
            "wo": np.ascontiguousarray(wo_sel).astype(BF),
        })
    return in_maps


def _mask_is_causal(attention_mask):
    m = np.asarray(attention_mask)
    if m.shape != (2, 1, S, S):
        return False
    neg = np.float32(np.finfo(np.float32).min)
    tri = np.tril(np.ones((S, S), dtype=bool))
    expect = np.where(tri, np.float32(0.0), neg)
    return all(np.array_equal(m[b, 0], expect) for b in range(m.shape[0]))


def _fallback(hidden_states, attention_mask, Wq, Wk, Wv, Wo, lambda_param):
    hs = hidden_states.astype(np.float32)
    lam = math.tanh(math.log1p(math.exp(float(lambda_param))))
    scaling = HD ** -0.5
    B = hs.shape[0]
    out = np.empty((B, S, H), dtype=np.float32)
    for b in range(B):
        q_all = (hs[b] @ Wq).reshape(S, 2 * NH, HD).transpose(1, 0, 2)
        k_all = (hs[b] @ Wk).reshape(S, 2 * NH, HD).transpose(1, 0, 2)
        v = (hs[b] @ Wv).reshape(S, NH, HD).transpose(1, 0, 2)
        acc = np.zeros((S, H), dtype=np.float32)
        for h in range(NH):
            s1 = q_all[h] @ k_all[h].T
            s2 = q_all[NH + h] @ k_all[NH + h].T
            sc = (s1 - lam * s2) * scaling + attention_mask[b, 0]
            sc -= sc.max(axis=-1, keepdims=True)
            p = np.exp(sc)
            p /= p.sum(axis=-1, keepdims=True)
            acc += (p @ v[h]) @ Wo[h * 64:(h + 1) * 64]
        out[b] = acc
    return out


def _run(inputs, trace=False):
    from concourse.bass_utils import run_bass_kernel_spmd

    hidden_states = np.asarray(inputs["hidden_states"], dtype=np.float32)
    attention_mask = np.asarray(inputs["attention_mask"], dtype=np.float32)
    Wq = np.asarray(inputs["Wq"], dtype=np.float32)
    Wk = np.asarray(inputs["Wk"], dtype=np.float32)
    Wv = np.asarray(inputs["Wv"], dtype=np.float32)
    Wo = np.asarray(inputs["Wo"], dtype=np.float32)
    lam_p = inputs["lambda_param"]

    if not _mask_is_causal(attention_mask):
        return _fallback(hidden_states, attention_mask, Wq, Wk, Wv, Wo, lam_p), None

    in_maps = _prep_in_maps(hidden_states, Wq, Wk, Wv, Wo, lam_p)
    nc = _get_nc()
    res = run_bass_kernel_spmd(nc, in_maps, list(range(8)), trace=trace)
    out = np.empty((2, S, H), dtype=np.float32)
    for b in range(2):
        acc = res.results[4 * b]["out"].astype(np.float32)
        for g in range(1, 4):
            acc = acc + res.results[4 * b + g]["out"].astype(np.float32)
        out[b] = acc
    return out, res


def kernel(**inputs):
    out, _ = _run(inputs, trace=False)
    return out


# revision 32
# speedup vs baseline: 1.0209x; 1.0162x over previous
"""Differential attention kernel for 8 Trainium2 NeuronCores.

Sharding: batch x head-group. Core c handles batch b = c//4, heads
[4g, 4g+4) with g = c%4. Each core computes Q/K/V projections for its
heads over the full sequence, causal differential attention, and its
partial O-projection; the host sums the 4 bf16 partials per batch.

Differential attention trick: score = (q1.k1 - lam*q2.k2) * scale is a
single K=128 matmul with stacked [q1*scale; -lam*scale*q2] and [k1; k2]
head vectors (scales folded into the projection weights on the host).

Softmax: scores are computed transposed (keys on partitions, queries
free), exp'd without max subtraction (inputs are bounded; exp is exact
to 2ULP on ACT), and the denominator comes for free from a ones-column
in V in the P@V matmul. Causality is applied structurally (upper blocks
skipped, diagonal blocks column-trimmed in the score matmul and zeroed
post-exp), which the host validates against the attention_mask input
before dispatch.

Emission order is engineered for engine overlap (the Tile scheduler is
a per-engine priority heap, priority = emission order, and PSUM pools
must coexist within 8 banks):
  A : QK proj heads 0,1        (pp 2x[128,2048] = 8 banks, DMA-paced)
  A2: V proj (3 banks) || QK proj head 2 (pp2 2x[128,512] = 2 banks)
  B : attention heads 0,1 (sc 4 + av 2 banks) || QK proj head 3 (2)
      - head-3 matmuls fill PE idle while ACT runs the softmax exp
  C : attention heads 2,3 || O-proj one chunk behind (op 2 banks)
"""
import math
from contextlib import ExitStack

import numpy as np
import ml_dtypes

S = 2048
H = 2048
NH = 16
HD = 64
NHC = 4          # heads per core
BF = ml_dtypes.bfloat16

_CACHED_NC = None


def _build_nc():
    import concourse.mybir as mybir
    import concourse.tile as tile
    from concourse import bacc

    bf16 = mybir.dt.bfloat16
    f32 = mybir.dt.float32
    AF = mybir.ActivationFunctionType

    nc = bacc.Bacc(None, target_bir_lowering=False)
    hT = nc.declare_dram_parameter("hT", [H, S], bf16, isOutput=False)
    wq = nc.declare_dram_parameter("wq", [H, NHC * 128], bf16, isOutput=False)
    wk = nc.declare_dram_parameter("wk", [H, NHC * 128], bf16, isOutput=False)
    wv = nc.declare_dram_parameter("wv", [H, NHC * 65], bf16, isOutput=False)
    wo = nc.declare_dram_parameter("wo", [128, 2, S], bf16, isOutput=False)
    out = nc.declare_dram_parameter("out", [S, H], bf16, isOutput=True)

    KT = H // 128    # 16 contraction tiles for projections
    NQ = S // 512    # 4 query chunks
    NS = S // 128    # 16 seq tiles

    with tile.TileContext(nc) as tc:
        with ExitStack() as ctx:
            # ---- persistent SBUF ----
            sb = ctx.enter_context(tc.tile_pool(name="sb", bufs=1))
            qk_sb = ctx.enter_context(tc.tile_pool(name="qk", bufs=1))
            ht_all = sb.tile([128, KT, S], bf16)          # hidden^T
            wq_sb = sb.tile([128, KT, NHC * 128], bf16)
            wk_sb = sb.tile([128, KT, NHC * 128], bf16)
            wv_sb = sb.tile([128, KT, NHC * 65], bf16)
            wo_sb = sb.tile([128, 2, S], bf16)            # head-pair stacked Wo rows
            qT = qk_sb.tile([128, NHC, S], bf16)          # [q1*s; -lam*s*q2] per head
            kT = qk_sb.tile([128, NHC, S], bf16)          # [k1; k2] per head
            v4 = qk_sb.tile([128, NS, NHC * 65], bf16)    # V tiles + ones cols
            avt = qk_sb.tile([128, 2, S], bf16)           # attn_out^T, head pairs stacked
            # warm the ACT exp table while DMAs stream in
            warm = sb.tile([1, 16], f32)
            nc.vector.memset(warm[:], 0.0)
            nc.scalar.activation(warm[:], warm[:], AF.Exp)

            # ---- input DMAs, k-ordered so the phase-A chains (k-outer)
            # start as soon as the first rows land. hT alternates across
            # both HWDGE queues (phase A is paced by hT arrival: each
            # 512 KB tile backs 1.9 us of PE work, one queue sustains
            # only ~175 GB/s). wq/wk ride the scalar queue; wv/wo are
            # only needed from phase A2 on and go on the slower gpsimd
            # SWDGE queue. ----
            nc.scalar.dma_start(out=wq_sb[:, 0, :], in_=wq[0:128, :])
            nc.scalar.dma_start(out=wk_sb[:, 0, :], in_=wk[0:128, :])
            for k in range(KT):
                rs = slice(k * 128, (k + 1) * 128)
                eng = nc.sync if k % 2 == 0 else nc.scalar
                eng.dma_start(out=ht_all[:, k, :], in_=hT[rs, :])
                if k > 0:
                    nc.scalar.dma_start(out=wq_sb[:, k, :], in_=wq[rs, :])
                    nc.scalar.dma_start(out=wk_sb[:, k, :], in_=wk[rs, :])
                nc.gpsimd.dma_start(out=wv_sb[:, k, :], in_=wv[rs, :])
            nc.gpsimd.dma_start(out=wo_sb[:], in_=wo[:, :, :])

            # ---- phase A: q/k projections for heads 0,1 (k-outer,
            # nj-inner: each k-step is 4x512 cols so the PE keeps pace
            # with the hT DMA stream) ----
            with tc.tile_pool(name="pjp", bufs=2, space="PSUM") as pjp:
                for h in range(2):
                    hs = slice(h * 128, (h + 1) * 128)
                    for w_sb, dst in ((wq_sb, qT), (wk_sb, kT)):
                        pp = pjp.tile([128, S], f32, tag="qkp")
                        for k in range(KT):
                            for nj in range(NQ):
                                nc.tensor.matmul(pp[:, nj * 512:(nj + 1) * 512],
                                                 lhsT=w_sb[:, k, hs],
                                                 rhs=ht_all[:, k, nj * 512:(nj + 1) * 512],
                                                 start=(k == 0), stop=(k == KT - 1))
                        for nj in range(NQ):
                            nc.vector.tensor_copy(dst[:, h, nj * 512:(nj + 1) * 512],
                                                  pp[:, nj * 512:(nj + 1) * 512])

            # 2-bank PSUM pool shared (same tag -> same slots, PSUM pool
            # lifetimes are LIFO) by the head-2/3 projection chains and
            # later by the O-projection tiles
            ch5 = ctx.enter_context(tc.tile_pool(name="ch5", bufs=2, space="PSUM"))

            # small-footprint q/k projection chain for heads 2,3 (hT is
            # fully resident by the time these run, so no DMA pacing
            # concerns; 1 PSUM bank per chain)
            def qk23_chain(h, w_sb, dst, nj):
                hs = slice(h * 128, (h + 1) * 128)
                qs = slice(nj * 512, (nj + 1) * 512)
                pq = ch5.tile([128, 512], f32, tag="ch512")
                for k in range(KT):
                    nc.tensor.matmul(pq[:], lhsT=w_sb[:, k, hs],
                                     rhs=ht_all[:, k, qs],
                                     start=(k == 0), stop=(k == KT - 1))
                nc.vector.tensor_copy(dst[:, h, qs], pq[:])

            # ---- phase A2: V projection (natural [S, 260] layout)
            # interleaved with head-2 q/k chains ----
            h2_chains = [(2, w_sb, dst, nj)
                         for w_sb, dst in ((wq_sb, qT), (wk_sb, kT))
                         for nj in range(NQ)]
            with tc.tile_pool(name="vjp", bufs=3, space="PSUM") as vjp:
                for st in range(NS):
                    vp = vjp.tile([128, NHC * 65], f32, tag="vp")
                    for k in range(KT):
                        nc.tensor.matmul(vp[:], lhsT=ht_all[:, k, st * 128:(st + 1) * 128],
                                         rhs=wv_sb[:, k, :],
                                         start=(k == 0), stop=(k == KT - 1))
                    nc.vector.tensor_copy(v4[:, st, :], vp[:])
                    for j in range(NHC):
                        nc.gpsimd.memset(v4[:, st, j * 65 + 64:j * 65 + 65], 1.0)
                    if st % 2 == 1:
                        h2_chains and qk23_chain(*h2_chains.pop(0))

            # ---- attention chunk for one head pair ----
            att_work = ctx.enter_context(tc.tile_pool(name="attw", bufs=3))
            nrm_work = ctx.enter_context(tc.tile_pool(name="nrmw", bufs=2))
            atp = tc.alloc_tile_pool(name="atp", bufs=1, space="PSUM")

            def attn_chunk(pair, nj):
                """Scores + exp + P@V for heads (2*pair, 2*pair+1) on
                query chunk nj. Returns the two av PSUM tiles."""
                qs = slice(nj * 512, (nj + 1) * 512)
                nblk = 4 * nj + 4
                pair_heads = (2 * pair, 2 * pair + 1)
                pav = {}
                for h in pair_heads:
                    pav[h] = atp.tile([65, 512], f32, tag=f"av{h % 2}",
                                      name=f"av{h}")
                # round-robin the two heads per key-block group so one
                # head's score matmuls hide the other head's exp
                for kg in range(nblk // 2):
                    scs, pts = {}, {}
                    # per-u start column: diagonal blocks (uu>=0) have
                    # their first 128*uu query columns fully masked, so
                    # both the score matmul and the exp skip them
                    cuts = [max(2 * kg + u - 4 * nj, 0) * 128 for u in range(2)]
                    for h in pair_heads:
                        sc = atp.tile([128, 1024], f32, tag=f"sc{h % 2}",
                                      name=f"sc{h}")
                        scs[h] = sc
                        for u in range(2):
                            ki = 2 * kg + u
                            c0 = cuts[u]
                            nc.tensor.matmul(sc[:, u * 512 + c0:(u + 1) * 512],
                                             lhsT=kT[:, h, ki * 128:(ki + 1) * 128],
                                             rhs=qT[:, h, nj * 512 + c0:(nj + 1) * 512],
                                             start=True, stop=True)
                    for h in pair_heads:
                        pt = att_work.tile([128, 1024], bf16, tag=f"pt{h % 2}",
                                           bufs=4, name=f"pt{h}")
                        pts[h] = pt
                        if cuts[1] == 0:
                            nc.scalar.activation(pt[:, cuts[0]:1024],
                                                 scs[h][:, cuts[0]:1024], AF.Exp)
                        else:
                            for u in range(2):
                                lo, hi = u * 512 + cuts[u], (u + 1) * 512
                                nc.scalar.activation(pt[:, lo:hi],
                                                     scs[h][:, lo:hi], AF.Exp)
                    for h in pair_heads:
                        pt = pts[h]
                        av = pav[h]
                        for u in range(2):
                            ki = 2 * kg + u
                            uu = ki - 4 * nj  # >=0 on diagonal blocks
                            if uu >= 0:
                                b0 = u * 512 + uu * 128
                                nc.gpsimd.affine_select(
                                    out=pt[:, b0:b0 + 128],
                                    in_=pt[:, b0:b0 + 128],
                                    compare_op=mybir.AluOpType.is_ge,
                                    fill=0.0,
                                    base=0,
                                    channel_multiplier=-1,
                                    pattern=[[1, 128]],
                                )
                                nc.tensor.matmul(av[:, uu * 128:512],
                                                 lhsT=v4[:, ki, h * 65:(h + 1) * 65],
                                                 rhs=pt[:, u * 512 + uu * 128:(u + 1) * 512],
                                                 start=(ki == 0), stop=(ki == nblk - 1))
                            else:
                                nc.tensor.matmul(av[:],
                                                 lhsT=v4[:, ki, h * 65:(h + 1) * 65],
                                                 rhs=pt[:, u * 512:(u + 1) * 512],
                                                 start=(ki == 0), stop=(ki == nblk - 1))
                return pav

            def norm_chunk(pair, nj, pav):
                """row 64 (even) / 63 (odd) of av is the softmax
                denominator; scale the 64 attn rows by 1/denom into avt.
                Reciprocal runs on a [128, 8] DMA-folded layout so all
                128 DVE lanes work instead of one."""
                qs = slice(nj * 512, (nj + 1) * 512)
                pair_heads = (2 * pair, 2 * pair + 1)
                dfold = nrm_work.tile([128, 8], f32, tag="dfold")
                araws = {}
                for j, h in enumerate(pair_heads):
                    den = nrm_work.tile([65, 512], f32, tag="den", bufs=4)
                    nc.vector.tensor_copy(den[64:65, :], pav[h][64:65, :])
                    nc.sync.dma_start(out=dfold[:, 4 * j:4 * j + 4], in_=den[64:65, :])
                    # evacuate the numerator too so the PSUM bank frees
                    # without waiting for the normalization chain
                    araw = nrm_work.tile([64, 512], bf16, tag=f"araw{h % 2}",
                                         bufs=2, name=f"araw{h}")
                    nc.vector.tensor_copy(araw[:], pav[h][0:64, :])
                    araws[h] = araw
                nc.vector.reciprocal(dfold[:], dfold[:])
                for j, h in enumerate(pair_heads):
                    rc0 = nrm_work.tile([1, 512], f32, tag="rc0")
                    nc.sync.dma_start(out=rc0[:], in_=dfold[:, 4 * j:4 * j + 4])
                    bcs = nrm_work.tile([64, 512], f32, tag=f"bcs{h % 2}",
                                        name=f"bcs{h}")
                    nc.gpsimd.partition_broadcast(bcs[:], rc0[:])
                    if h % 2:
                        om = nrm_work.tile([64, 512], bf16, tag="om", bufs=2)
                        nc.vector.tensor_mul(om[:], araws[h][:], bcs[:])
                        nc.sync.dma_start(out=avt[64:128, pair, qs], in_=om[:])
                    else:
                        nc.vector.tensor_mul(avt[0:64, pair, qs], araws[h][:], bcs[:])

            # ---- phase B: attention pair 0, head-3 q/k chains woven in
            # between chunks (they fill PE idle while ACT runs exp) ----
            h3_chains = [(3, w_sb, dst, nj)
                         for w_sb, dst in ((wq_sb, qT), (wk_sb, kT))
                         for nj in range(NQ)]
            for nj in range(NQ):
                pav = attn_chunk(0, nj)
                norm_chunk(0, nj, pav)
                for _ in range(2):
                    h3_chains and qk23_chain(*h3_chains.pop(0))

            # ---- phase C: attention pair 1 with O-projection trailing
            # one chunk behind ----
            oout_sb = ctx.enter_context(tc.tile_pool(name="oout", bufs=5))

            def oproj_chunk(nj, pool, tail=False):
                for ti, qi in enumerate(range(4 * nj, 4 * nj + 4)):
                    for nch in range(NQ):
                        op = pool.tile([128, 512], f32, tag="ch512")
                        for p in range(2):
                            nc.tensor.matmul(op[:],
                                             lhsT=avt[:, p, qi * 128:(qi + 1) * 128],
                                             rhs=wo_sb[:, p, nch * 512:(nch + 1) * 512],
                                             start=(p == 0), stop=(p == 1))
                        ot = oout_sb.tile([128, 512], bf16, tag="ot")
                        # DVE is the busiest engine in phase C but ACT
                        # carries the exps, so only skim its spare duty
                        # (1-in-4) until the exps are done (tail: 1-in-2)
                        if nch % (2 if tail else 4) == 1:
                            nc.scalar.copy(ot[:], op[:])
                        else:
                            nc.vector.tensor_copy(ot[:], op[:])
                        deng = nc.gpsimd if tail and nch % 2 == 0 else nc.sync
                        deng.dma_start(
                            out=out[qi * 128:(qi + 1) * 128, nch * 512:(nch + 1) * 512],
                            in_=ot[:])

            for nj in range(NQ):
                pav = attn_chunk(1, nj)
                norm_chunk(1, nj, pav)
                if nj > 0:
                    oproj_chunk(nj - 1, ch5)
            # the attention pools are done: hand their banks to a deeper
            # O-proj pool so the tail isn't throttled by 2-slot rotation
            atp.release()
            otp = tc.alloc_tile_pool(name="otp", bufs=5, space="PSUM")
            oproj_chunk(NQ - 1, otp, tail=True)
            otp.release()
    return nc


def _get_nc():
    global _CACHED_NC
    if _CACHED_NC is None:
        nc = _build_nc()
        if not nc.is_finalized():
            nc.finalize()
        _CACHED_NC = nc
    return _CACHED_NC


def _prep_in_maps(hidden_states, Wq, Wk, Wv, Wo, lambda_param):
    lam = math.tanh(math.log1p(math.exp(float(lambda_param))))
    scale = HD ** -0.5
    in_maps = []
    hTb = [np.ascontiguousarray(hidden_states[b].T).astype(BF) for b in range(2)]
    for core in range(8):
        b, g = divmod(core, 4)
        heads = range(NHC * g, NHC * g + NHC)
        wq_cols, wk_cols = [], []
        for h in heads:
            wq_cols.append(Wq[:, h * 64:(h + 1) * 64] * scale)
            wq_cols.append(Wq[:, (NH + h) * 64:(NH + h + 1) * 64] * (-lam * scale))
            wk_cols.append(Wk[:, h * 64:(h + 1) * 64])
            wk_cols.append(Wk[:, (NH + h) * 64:(NH + h + 1) * 64])
        wv_pad = np.zeros((H, NHC * 65), dtype=np.float32)
        for j, h in enumerate(heads):
            wv_pad[:, j * 65:j * 65 + 64] = Wv[:, h * 64:(h + 1) * 64]
        heads = list(heads)
        wo_sel = np.zeros((128, 2, S), dtype=np.float32)  # head-pair stacked rows
        for p in range(2):
            h0, h1 = heads[2 * p], heads[2 * p + 1]
            wo_sel[0:64, p] = Wo[h0 * 64:(h0 + 1) * 64, :]
            wo_sel[64:128, p] = Wo[h1 * 64:(h1 + 1) * 64, :]
        in_maps.append({
            "hT": hTb[b],
            "wq": np.concatenate(wq_cols, axis=1).astype(BF),
            "wk": np.concatenate(wk_cols, axis=1).astype(BF),
            "wv": wv_pad.astype(BF),
            "wo": np.ascontiguousarray(wo_sel).astype(BF),
        })
    return in_maps


def _mask_is_causal(attention_mask):
    m = np.asarray(attention_mask)
    if m.shape != (2, 1, S, S):
        return False
    neg = np.float32(np.finfo(np.float32).min)
    tri = np.tril(np.ones((S, S), dtype=bool))
    expect = np.where(tri, np.float32(0.0), neg)
    return all(np.array_equal(m[b, 0], expect) for b in range(m.shape[0]))


def _fallback(hidden_states, attention_mask, Wq, Wk, Wv, Wo, lambda_param):
    hs = hidden_states.astype(np.float32)
    lam = math.tanh(math.log1p(math.exp(float(lambda_param))))
    scaling = HD ** -0.5
    B = hs.shape[0]
    out = np.empty((B, S, H), dtype=np.float32)
    for b in range(B):
        q_all = (hs[b] @ Wq).reshape(S, 2 * NH, HD).transpose(1, 0, 2)
        k_all = (hs[b] @ Wk).reshape(S, 2 * NH, HD).transpose(1, 0, 2)
        v = (hs[b] @ Wv).reshape(S, NH, HD).transpose(1, 0, 2)
        acc = np.zeros((S, H), dtype=np.float32)
        for h in range(NH):
            s1 = q_all[h] @ k_all[h].T
            s2 = q_all[NH + h] @ k_all[NH + h].T
            sc = (s1 - lam * s2) * scaling + attention_mask[b, 0]
            sc -= sc.max(axis=-1, keepdims=True)
            p = np.exp(sc)
            p /= p.sum(axis=-1, keepdims=True)
            acc += (p @ v[h]) @ Wo[h * 64:(h + 1) * 64]
        out[b] = acc
    return out


def _run(inputs, trace=False):
    from concourse.bass_utils import run_bass_kernel_spmd

    hidden_states = np.asarray(inputs["hidden_states"], dtype=np.float32)
    attention_mask = np.asarray(inputs["attention_mask"], dtype=np.float32)
    Wq = np.asarray(inputs["Wq"], dtype=np.float32)
    Wk = np.asarray(inputs["Wk"], dtype=np.float32)
    Wv = np.asarray(inputs["Wv"], dtype=np.float32)
    Wo = np.asarray(inputs["Wo"], dtype=np.float32)
    lam_p = inputs["lambda_param"]

    if not _mask_is_causal(attention_mask):
        return _fallback(hidden_states, attention_mask, Wq, Wk, Wv, Wo, lam_p), None

    in_maps = _prep_in_maps(hidden_states, Wq, Wk, Wv, Wo, lam_p)
    nc = _get_nc()
    res = run_bass_kernel_spmd(nc, in_maps, list(range(8)), trace=trace)
    out = np.empty((2, S, H), dtype=np.float32)
    for b in range(2):
        acc = res.results[4 * b]["out"].astype(np.float32)
        for g in range(1, 4):
            acc = acc + res.results[4 * b + g]["out"].astype(np.float32)
        out[b] = acc
    return out, res


def kernel(**inputs):
    out, _ = _run(inputs, trace=False)
    return out


# revision 34
# speedup vs baseline: 1.0673x; 1.0454x over previous
"""Differential attention kernel for 8 Trainium2 NeuronCores.

Sharding: batch x head-group. Core c handles batch b = c//4, heads
[4g, 4g+4) with g = c%4. Each core computes Q/K/V projections for its
heads over the full sequence, causal differential attention, and its
partial O-projection; the host sums the 4 bf16 partials per batch.

Differential attention trick: score = (q1.k1 - lam*q2.k2) * scale is a
single K=128 matmul with stacked [q1*scale; -lam*scale*q2] and [k1; k2]
head vectors (scales folded into the projection weights on the host).

Softmax: scores are computed transposed (keys on partitions, queries
free), exp'd without max subtraction (inputs are bounded; exp is exact
to 2ULP on ACT), and the denominator comes for free from a ones-column
in V in the P@V matmul. Causality is applied structurally (upper blocks
skipped, diagonal blocks column-trimmed in the score matmul and zeroed
post-exp), which the host validates against the attention_mask input
before dispatch.

Emission order is engineered for engine overlap (the Tile scheduler is
a per-engine priority heap, priority = emission order, and PSUM pools
must coexist within 8 banks):
  A : QK proj heads 0,1        (pp 2x[128,2048] = 8 banks, DMA-paced)
  A2: V proj (3 banks) || QK proj head 2 (pp2 2x[128,512] = 2 banks)
  B : attention heads 0,1 (sc 4 + av 2 banks) || QK proj head 3 (2)
      - head-3 matmuls fill PE idle while ACT runs the softmax exp
  C : attention heads 2,3 || O-proj one chunk behind (op 2 banks)
"""
import math
from contextlib import ExitStack

import numpy as np
import ml_dtypes

S = 2048
H = 2048
NH = 16
HD = 64
NHC = 4          # heads per core
BF = ml_dtypes.bfloat16

_CACHED_NC = None


def _build_nc():
    import concourse.mybir as mybir
    import concourse.tile as tile
    from concourse import bacc

    bf16 = mybir.dt.bfloat16
    f32 = mybir.dt.float32
    AF = mybir.ActivationFunctionType

    nc = bacc.Bacc(None, target_bir_lowering=False)
    hT = nc.declare_dram_parameter("hT", [H, S], bf16, isOutput=False)
    wq = nc.declare_dram_parameter("wq", [H, NHC * 128], bf16, isOutput=False)
    wk = nc.declare_dram_parameter("wk", [H, NHC * 128], bf16, isOutput=False)
    wv = nc.declare_dram_parameter("wv", [H, NHC * 65], bf16, isOutput=False)
    wo = nc.declare_dram_parameter("wo", [128, 2, S], bf16, isOutput=False)
    out = nc.declare_dram_parameter("out", [S, H], bf16, isOutput=True)

    KT = H // 128    # 16 contraction tiles for projections
    NQ = S // 512    # 4 query chunks
    NS = S // 128    # 16 seq tiles

    with tile.TileContext(nc) as tc:
        with ExitStack() as ctx:
            # ---- persistent SBUF ----
            sb = ctx.enter_context(tc.tile_pool(name="sb", bufs=1))
            qk_sb = ctx.enter_context(tc.tile_pool(name="qk", bufs=1))
            ht_all = sb.tile([128, KT, S], bf16)          # hidden^T
            wq_sb = sb.tile([128, KT, NHC * 128], bf16)
            wk_sb = sb.tile([128, KT, NHC * 128], bf16)
            wv_sb = sb.tile([128, KT, NHC * 65], bf16)
            wo_sb = sb.tile([128, 2, S], bf16)            # head-pair stacked Wo rows
            qT = qk_sb.tile([128, NHC, S], bf16)          # [q1*s; -lam*s*q2] per head
            kT = qk_sb.tile([128, NHC, S], bf16)          # [k1; k2] per head
            v4 = qk_sb.tile([128, NS, NHC * 65], bf16)    # V tiles + ones cols
            avt = qk_sb.tile([128, 2, S], bf16)           # attn_out^T, head pairs stacked
            # warm the ACT exp table while DMAs stream in
            warm = sb.tile([1, 16], f32)
            nc.vector.memset(warm[:], 0.0)
            nc.scalar.activation(warm[:], warm[:], AF.Exp)

            # ---- input DMAs, k-ordered so the phase-A chains (k-outer)
            # start as soon as the first rows land. hT alternates across
            # both HWDGE queues (phase A is paced by hT arrival: each
            # 512 KB tile backs 1.9 us of PE work, one queue sustains
            # only ~175 GB/s). wq/wk ride the scalar queue; wv/wo are
            # only needed from phase A2 on and go on the slower gpsimd
            # SWDGE queue. ----
            nc.scalar.dma_start(out=wq_sb[:, 0, :], in_=wq[0:128, :])
            nc.scalar.dma_start(out=wk_sb[:, 0, :], in_=wk[0:128, :])
            for k in range(KT):
                rs = slice(k * 128, (k + 1) * 128)
                eng = nc.sync if k % 2 == 0 else nc.scalar
                eng.dma_start(out=ht_all[:, k, :], in_=hT[rs, :])
                if k > 0:
                    nc.scalar.dma_start(out=wq_sb[:, k, :], in_=wq[rs, :])
                    nc.scalar.dma_start(out=wk_sb[:, k, :], in_=wk[rs, :])
            # wv/wo aren't needed until phase A2/C: put them at the TAIL
            # of the scalar HWDGE queue so they don't steal HBM bandwidth
            # from the hT stream that paces phase A
            for k in range(KT):
                nc.scalar.dma_start(out=wv_sb[:, k, :],
                                    in_=wv[k * 128:(k + 1) * 128, :])
            nc.scalar.dma_start(out=wo_sb[:], in_=wo[:, :, :])

            # ---- phase A: q/k projections for heads 0,1 (k-outer,
            # nj-inner: each k-step is 4x512 cols so the PE keeps pace
            # with the hT DMA stream) ----
            with tc.tile_pool(name="pjp", bufs=2, space="PSUM") as pjp:
                for h in range(2):
                    hs = slice(h * 128, (h + 1) * 128)
                    for w_sb, dst in ((wq_sb, qT), (wk_sb, kT)):
                        pp = pjp.tile([128, S], f32, tag="qkp")
                        for k in range(KT):
                            for nj in range(NQ):
                                nc.tensor.matmul(pp[:, nj * 512:(nj + 1) * 512],
                                                 lhsT=w_sb[:, k, hs],
                                                 rhs=ht_all[:, k, nj * 512:(nj + 1) * 512],
                                                 start=(k == 0), stop=(k == KT - 1))
                        for nj in range(NQ):
                            nc.vector.tensor_copy(dst[:, h, nj * 512:(nj + 1) * 512],
                                                  pp[:, nj * 512:(nj + 1) * 512])

            # 2-bank PSUM pool shared (same tag -> same slots, PSUM pool
            # lifetimes are LIFO) by the head-2/3 projection chains and
            # later by the O-projection tiles
            ch5 = ctx.enter_context(tc.tile_pool(name="ch5", bufs=2, space="PSUM"))

            # small-footprint q/k projection chain for heads 2,3 (hT is
            # fully resident by the time these run, so no DMA pacing
            # concerns; 1 PSUM bank per chain)
            def qk23_chain(h, w_sb, dst, nj):
                hs = slice(h * 128, (h + 1) * 128)
                qs = slice(nj * 512, (nj + 1) * 512)
                pq = ch5.tile([128, 512], f32, tag="ch512")
                for k in range(KT):
                    nc.tensor.matmul(pq[:], lhsT=w_sb[:, k, hs],
                                     rhs=ht_all[:, k, qs],
                                     start=(k == 0), stop=(k == KT - 1))
                nc.vector.tensor_copy(dst[:, h, qs], pq[:])

            # ---- phase A2: V projection (natural [S, 260] layout)
            # interleaved with head-2 q/k chains ----
            h2_chains = [(2, w_sb, dst, nj)
                         for w_sb, dst in ((wq_sb, qT), (wk_sb, kT))
                         for nj in range(NQ)]
            with tc.tile_pool(name="vjp", bufs=3, space="PSUM") as vjp:
                for st in range(NS):
                    vp = vjp.tile([128, NHC * 65], f32, tag="vp")
                    for k in range(KT):
                        nc.tensor.matmul(vp[:], lhsT=ht_all[:, k, st * 128:(st + 1) * 128],
                                         rhs=wv_sb[:, k, :],
                                         start=(k == 0), stop=(k == KT - 1))
                    nc.vector.tensor_copy(v4[:, st, :], vp[:])
                    for j in range(NHC):
                        nc.gpsimd.memset(v4[:, st, j * 65 + 64:j * 65 + 65], 1.0)
                    if st % 2 == 1:
                        h2_chains and qk23_chain(*h2_chains.pop(0))

            # ---- attention chunk for one head pair ----
            att_work = ctx.enter_context(tc.tile_pool(name="attw", bufs=3))
            nrm_work = ctx.enter_context(tc.tile_pool(name="nrmw", bufs=2))
            atp = tc.alloc_tile_pool(name="atp", bufs=1, space="PSUM")

            def attn_chunk(pair, nj):
                """Scores + exp + P@V for heads (2*pair, 2*pair+1) on
                query chunk nj. Returns the two av PSUM tiles."""
                qs = slice(nj * 512, (nj + 1) * 512)
                nblk = 4 * nj + 4
                pair_heads = (2 * pair, 2 * pair + 1)
                pav = {}
                for h in pair_heads:
                    pav[h] = atp.tile([65, 512], f32, tag=f"av{h % 2}",
                                      name=f"av{h}")
                # round-robin the two heads per key-block group so one
                # head's score matmuls hide the other head's exp
                for kg in range(nblk // 2):
                    scs, pts = {}, {}
                    # per-u start column: diagonal blocks (uu>=0) have
                    # their first 128*uu query columns fully masked, so
                    # both the score matmul and the exp skip them
                    cuts = [max(2 * kg + u - 4 * nj, 0) * 128 for u in range(2)]
                    for h in pair_heads:
                        sc = atp.tile([128, 1024], f32, tag=f"sc{h % 2}",
                                      name=f"sc{h}")
                        scs[h] = sc
                        for u in range(2):
                            ki = 2 * kg + u
                            c0 = cuts[u]
                            nc.tensor.matmul(sc[:, u * 512 + c0:(u + 1) * 512],
                                             lhsT=kT[:, h, ki * 128:(ki + 1) * 128],
                                             rhs=qT[:, h, nj * 512 + c0:(nj + 1) * 512],
                                             start=True, stop=True)
                    for h in pair_heads:
                        pt = att_work.tile([128, 1024], bf16, tag=f"pt{h % 2}",
                                           bufs=4, name=f"pt{h}")
                        pts[h] = pt
                        # one exp per head per key-group: it may read
                        # stale PSUM in the trimmed fully-masked zones
                        # (cols [u*512, u*512+cuts[u])), but those pt
                        # columns are never consumed by the P@V matmul
                        nc.scalar.activation(pt[:, cuts[0]:1024],
                                             scs[h][:, cuts[0]:1024], AF.Exp)
                    for h in pair_heads:
                        pt = pts[h]
                        av = pav[h]
                        for u in range(2):
                            ki = 2 * kg + u
                            uu = ki - 4 * nj  # >=0 on diagonal blocks
                            if uu >= 0:
                                b0 = u * 512 + uu * 128
                                nc.gpsimd.affine_select(
                                    out=pt[:, b0:b0 + 128],
                                    in_=pt[:, b0:b0 + 128],
                                    compare_op=mybir.AluOpType.is_ge,
                                    fill=0.0,
                                    base=0,
                                    channel_multiplier=-1,
                                    pattern=[[1, 128]],
                                )
                                nc.tensor.matmul(av[:, uu * 128:512],
                                                 lhsT=v4[:, ki, h * 65:(h + 1) * 65],
                                                 rhs=pt[:, u * 512 + uu * 128:(u + 1) * 512],
                                                 start=(ki == 0), stop=(ki == nblk - 1))
                            else:
                                nc.tensor.matmul(av[:],
                                                 lhsT=v4[:, ki, h * 65:(h + 1) * 65],
                                                 rhs=pt[:, u * 512:(u + 1) * 512],
                                                 start=(ki == 0), stop=(ki == nblk - 1))
                return pav

            def norm_chunk(pair, nj, pav):
                """row 64 (even) / 63 (odd) of av is the softmax
                denominator; scale the 64 attn rows by 1/denom into avt.
                Reciprocal runs on a [128, 8] DMA-folded layout so all
                128 DVE lanes work instead of one."""
                qs = slice(nj * 512, (nj + 1) * 512)
                pair_heads = (2 * pair, 2 * pair + 1)
                dfold = nrm_work.tile([128, 8], f32, tag="dfold")
                araws = {}
                for j, h in enumerate(pair_heads):
                    den = nrm_work.tile([65, 512], f32, tag="den", bufs=4)
                    nc.vector.tensor_copy(den[64:65, :], pav[h][64:65, :])
                    nc.sync.dma_start(out=dfold[:, 4 * j:4 * j + 4], in_=den[64:65, :])
                    # evacuate the numerator too so the PSUM bank frees
                    # without waiting for the normalization chain
                    araw = nrm_work.tile([64, 512], bf16, tag=f"araw{h % 2}",
                                         bufs=2, name=f"araw{h}")
                    nc.vector.tensor_copy(araw[:], pav[h][0:64, :])
                    araws[h] = araw
                nc.vector.reciprocal(dfold[:], dfold[:])
                for j, h in enumerate(pair_heads):
                    rc0 = nrm_work.tile([1, 512], f32, tag="rc0")
                    nc.sync.dma_start(out=rc0[:], in_=dfold[:, 4 * j:4 * j + 4])
                    bcs = nrm_work.tile([64, 512], f32, tag=f"bcs{h % 2}",
                                        name=f"bcs{h}")
                    nc.gpsimd.partition_broadcast(bcs[:], rc0[:])
                    if h % 2:
                        om = nrm_work.tile([64, 512], bf16, tag="om", bufs=2)
                        nc.vector.tensor_mul(om[:], araws[h][:], bcs[:])
                        nc.sync.dma_start(out=avt[64:128, pair, qs], in_=om[:])
                    else:
                        nc.vector.tensor_mul(avt[0:64, pair, qs], araws[h][:], bcs[:])

            # ---- phase B: attention pair 0, head-3 q/k chains woven in
            # between chunks (they fill PE idle while ACT runs exp) ----
            h3_chains = [(3, w_sb, dst, nj)
                         for w_sb, dst in ((wq_sb, qT), (wk_sb, kT))
                         for nj in range(NQ)]
            for nj in range(NQ):
                pav = attn_chunk(0, nj)
                norm_chunk(0, nj, pav)
                for _ in range(2):
                    h3_chains and qk23_chain(*h3_chains.pop(0))

            # ---- phase C: attention pair 1 with O-projection trailing
            # one chunk behind ----
            oout_sb = ctx.enter_context(tc.tile_pool(name="oout", bufs=5))

            def oproj_chunk(nj, pool, tail=False):
                for ti, qi in enumerate(range(4 * nj, 4 * nj + 4)):
                    for nch in range(NQ):
                        op = pool.tile([128, 512], f32, tag="ch512")
                        for p in range(2):
                            nc.tensor.matmul(op[:],
                                             lhsT=avt[:, p, qi * 128:(qi + 1) * 128],
                                             rhs=wo_sb[:, p, nch * 512:(nch + 1) * 512],
                                             start=(p == 0), stop=(p == 1))
                        ot = oout_sb.tile([128, 512], bf16, tag="ot")
                        # DVE is the busiest engine in phase C but ACT
                        # carries the exps, so only skim its spare duty
                        # (1-in-4) until the exps are done (tail: 1-in-2)
                        if nch % (2 if tail else 4) == 1:
                            nc.scalar.copy(ot[:], op[:])
                        else:
                            nc.vector.tensor_copy(ot[:], op[:])
                        deng = nc.gpsimd if tail and nch % 2 == 0 else nc.sync
                        deng.dma_start(
                            out=out[qi * 128:(qi + 1) * 128, nch * 512:(nch + 1) * 512],
                            in_=ot[:])

            for nj in range(NQ):
                pav = attn_chunk(1, nj)
                norm_chunk(1, nj, pav)
                if nj > 0:
                    oproj_chunk(nj - 1, ch5)
            # the attention pools are done: hand their banks to a deeper
            # O-proj pool so the tail isn't throttled by 2-slot rotation
            atp.release()
            otp = tc.alloc_tile_pool(name="otp", bufs=5, space="PSUM")
            oproj_chunk(NQ - 1, otp, tail=True)
            otp.release()
    return nc


def _get_nc():
    global _CACHED_NC
    if _CACHED_NC is None:
        nc = _build_nc()
        if not nc.is_finalized():
            nc.finalize()
        _CACHED_NC = nc
    return _CACHED_NC


def _prep_in_maps(hidden_states, Wq, Wk, Wv, Wo, lambda_param):
    lam = math.tanh(math.log1p(math.exp(float(lambda_param))))
    scale = HD ** -0.5
    in_maps = []
    hTb = [np.ascontiguousarray(hidden_states[b].T).astype(BF) for b in range(2)]
    for core in range(8):
        b, g = divmod(core, 4)
        heads = range(NHC * g, NHC * g + NHC)
        wq_cols, wk_cols = [], []
        for h in heads:
            wq_cols.append(Wq[:, h * 64:(h + 1) * 64] * scale)
            wq_cols.append(Wq[:, (NH + h) * 64:(NH + h + 1) * 64] * (-lam * scale))
            wk_cols.append(Wk[:, h * 64:(h + 1) * 64])
            wk_cols.append(Wk[:, (NH + h) * 64:(NH + h + 1) * 64])
        wv_pad = np.zeros((H, NHC * 65), dtype=np.float32)
        for j, h in enumerate(heads):
            wv_pad[:, j * 65:j * 65 + 64] = Wv[:, h * 64:(h + 1) * 64]
        heads = list(heads)
        wo_sel = np.zeros((128, 2, S), dtype=np.float32)  # head-pair stacked rows
        for p in range(2):
            h0, h1 = heads[2 * p], heads[2 * p + 1]
            wo_sel[0:64, p] = Wo[h0 * 64:(h0 + 1) * 64, :]
            wo_sel[64:128, p] = Wo[h1 * 64:(h1 + 1) * 64, :]
        in_maps.append({
            "hT": hTb[b],
            "wq": np.concatenate(wq_cols, axis=1).astype(BF),
            "wk": np.concatenate(wk_cols, axis=1).astype(BF),
            "wv": wv_pad.astype(BF),
            "wo": np.ascontiguousarray(wo_sel).astype(BF),
        })
    return in_maps


def _mask_is_causal(attention_mask):
    m = np.asarray(attention_mask)
    if m.shape != (2, 1, S, S):
        return False
    neg = np.float32(np.finfo(np.float32).min)
    tri = np.tril(np.ones((S, S), dtype=bool))
    expect = np.where(tri, np.float32(0.0), neg)
    return all(np.array_equal(m[b, 0], expect) for b in range(m.shape[0]))


def _fallback(hidden_states, attention_mask, Wq, Wk, Wv, Wo, lambda_param):
    hs = hidden_states.astype(np.float32)
    lam = math.tanh(math.log1p(math.exp(float(lambda_param))))
    scaling = HD ** -0.5
    B = hs.shape[0]
    out = np.empty((B, S, H), dtype=np.float32)
    for b in range(B):
        q_all = (hs[b] @ Wq).reshape(S, 2 * NH, HD).transpose(1, 0, 2)
        k_all = (hs[b] @ Wk).reshape(S, 2 * NH, HD).transpose(1, 0, 2)
        v = (hs[b] @ Wv).reshape(S, NH, HD).transpose(1, 0, 2)
        acc = np.zeros((S, H), dtype=np.float32)
        for h in range(NH):
            s1 = q_all[h] @ k_all[h].T
            s2 = q_all[NH + h] @ k_all[NH + h].T
            sc = (s1 - lam * s2) * scaling + attention_mask[b, 0]
            sc -= sc.max(axis=-1, keepdims=True)
            p = np.exp(sc)
            p /= p.sum(axis=-1, keepdims=True)
            acc += (p @ v[h]) @ Wo[h * 64:(h + 1) * 64]
        out[b] = acc
    return out


def _run(inputs, trace=False):
    from concourse.bass_utils import run_bass_kernel_spmd

    hidden_states = np.asarray(inputs["hidden_states"], dtype=np.float32)
    attention_mask = np.asarray(inputs["attention_mask"], dtype=np.float32)
    Wq = np.asarray(inputs["Wq"], dtype=np.float32)
    Wk = np.asarray(inputs["Wk"], dtype=np.float32)
    Wv = np.asarray(inputs["Wv"], dtype=np.float32)
    Wo = np.asarray(inputs["Wo"], dtype=np.float32)
    lam_p = inputs["lambda_param"]

    if not _mask_is_causal(attention_mask):
        return _fallback(hidden_states, attention_mask, Wq, Wk, Wv, Wo, lam_p), None

    in_maps = _prep_in_maps(hidden_states, Wq, Wk, Wv, Wo, lam_p)
    nc = _get_nc()
    res = run_bass_kernel_spmd(nc, in_maps, list(range(8)), trace=trace)
    out = np.empty((2, S, H), dtype=np.float32)
    for b in range(2):
        acc = res.results[4 * b]["out"].astype(np.float32)
        for g in range(1, 4):
            acc = acc + res.results[4 * b + g]["out"].astype(np.float32)
        out[b] = acc
    return out, res


def kernel(**inputs):
    out, _ = _run(inputs, trace=False)
    return out
